# revision 9
# baseline (speedup 1.0000x reference)
"""LocallyConnectedXYZLayer Trainium2 kernel (v2).

out[n,c,h,w] = sum_{dy,dx in 5x5} exp(-|xyz(n,:,h+dy-2,w+dx-2)-xyz(n,:,h,w)|^2/2)
               * (softmax*mask)(n,c,h+dy-2,w+dx-2)        (zero-padded)

Sharding: 8 cores = (batch n = core//2) x (W half = core%2).
Per-core layout: partitions = 2 w-chunks x 64 h rows; free dims carry
(dy, dx, c, w).  dy window shifts are host-baked into per-partition rows;
dx shifts are free-dim slices, duplicated at +0/+1 (parity e) so every
strided slice stays 4-byte aligned and the DVE runs tensor_tensor in
2x bf16 mode throughout.

Engine split per step (8 steps of 64 interior w per chunk):
  DVE   : diff = xyz_shift - xyz_center (2 merged subs), d2 = sum of
          squares (2 adds), t = g * sm (10 merged muls, c broadcast via
          stride-0 dim).
  ACT   : Square(diff), Exp(-d2/2), PSUM->SBUF output drains (all three
          live in the exp_and_others table set -> one table load).
  PE    : accumulates the 25 shifted products into PSUM with
          matmul(Identity, t_k) -- the adds cost nothing on DVE; the
          center term (g==1) is accumulated straight from sm, skipping
          its multiply.  fp32 PSUM accumulation, bf16 output store.
mask is folded into softmax on the host.
"""

import sys
from contextlib import ExitStack

import numpy as np

sys.path.insert(0, "/opt/trn_rl_repo")

import ml_dtypes  # noqa: E402

import concourse.bass as bass  # noqa: E402
from concourse import mybir  # noqa: E402
from concourse.bass_utils import run_bass_kernel_spmd  # noqa: E402

BF16 = ml_dtypes.bfloat16

N, C, H, W = 4, 20, 64, 2048
KH = KW = 5
PAD = 2
HH = H + 2 * PAD  # 68 padded rows
WCORE = W // 2  # 1024 interior w per core
NSTEP = 8
WS = WCORE // (2 * NSTEP)  # 64 interior w per (step, chunk)
WX = WS + 2 * PAD  # 68 w extent (halo 2 each side)

# per-partition element pitches
XP = 2 * KH * 3 * WX          # xyz tile pitch (2040)
SP = 2 * KH * C * WX          # sm tile pitch (27200)
SE = KH * C * WX              # sm parity block (6800)
DP = KH * 3 * KW * WS         # diff pitch (4800)
GP = KH * KW * WS             # d2/g pitch (1600)
TP = KW * C * WS              # t pitch (6400)
OP = C * WS                   # out pitch (1280)

_CACHE = {}


def _build_nc():
    """Raw-Bass program; cross-engine sync is standalone wait_ge
    instructions plus one then_inc per producer op (walrus allows at most
    one sync command per instruction)."""
    nc = bass.Bass("TRN2", target_bir_lowering=False, debug=False)
    bf = mybir.dt.bfloat16
    f32 = mybir.dt.float32
    sm_d = nc.dram_tensor("sm_in", [NSTEP, 128, 2, KH, C, WX], bf,
                          kind="ExternalInput")
    xyz_d = nc.dram_tensor("xyz_in", [NSTEP, 128, 2, KH, 3, WX], bf,
                           kind="ExternalInput")
    id_d = nc.dram_tensor("ident_in", [128, 128], bf, kind="ExternalInput")
    out_d = nc.dram_tensor("out_d", [NSTEP, 128, C, WS], bf,
                           kind="ExternalOutput")

    def sb(name, shape, dt):
        return nc.alloc_sbuf_tensor(name, list(shape), dt)

    xyz_t = [sb(f"xyz{i}", [128, 2, KH, 3, WX], bf) for i in range(2)]
    sm_t = [sb(f"sm{i}", [128, 2, KH, C, WX], bf) for i in range(2)]
    diff_t = sb("diff", [128, KH, 3, KW, WS], bf)
    sq_t = sb("sq", [128, KH, 3, KW, WS], bf)
    d2_t = [sb(f"d2_{i}", [128, KH, KW, WS], bf) for i in range(2)]
    g5_t = [sb(f"g5_{i}", [128, KH, KW, WS], bf) for i in range(2)]
    t_t = [sb(f"t{i}", [128, KW, C, WS], bf) for i in range(3)]
    out_t = [sb(f"out{i}", [128, C, WS], bf) for i in range(2)]
    id_t = sb("ident", [128, 128], bf)
    ps_t = [nc.alloc_psum_tensor(f"ps{i}", [128, OP], f32) for i in range(2)]

    ADD, MULT, SUB = (mybir.AluOpType.add, mybir.AluOpType.mult,
                      mybir.AluOpType.subtract)

    # column tiles for the PE/PSUM accumulation (c,w flattened)
    CT = [(0, 512), (512, 512), (1024, 256)]
    # dx slots per (dy, parity): even slots from parity-0 data, odd from
    # parity-1; dy==2 drops the center (dx==2) from the even list.
    def dxs_even(dy):
        return (0, 4) if dy == 2 else (0, 2, 4)

    # DVE ops per step block: 10 muls, 6 subs + 2 d2adds (for s+1).
    # Block positions: muls dy0 1-2, dy1 3-4, subs 5-10, dy2 11-12,
    # dy3 13-14, d2a 15-16, dy4 17-18 -- exp(s+1) overlaps the dy4
    # muls instead of stalling the next step.  Prologue: 6 subs + 2 d2a.
    DVE_STEP = 18
    DVE_PRO = 8

    def dve_at(s, pos):
        # semaphore value after `pos` ops of step-s block
        return DVE_PRO + DVE_STEP * s + pos

    with ExitStack() as ctx:
        load_sem = ctx.enter_context(nc.semaphore("load_sem"))
        sm0_sem = ctx.enter_context(nc.semaphore("sm0_sem"))
        sm1_sem = ctx.enter_context(nc.semaphore("sm1_sem"))
        id_sem = ctx.enter_context(nc.semaphore("id_sem"))
        store_sem = ctx.enter_context(nc.semaphore("store_sem"))
        dve_sem = ctx.enter_context(nc.semaphore("dve_sem"))
        act_sem = ctx.enter_context(nc.semaphore("act_sem"))
        drain_sem = ctx.enter_context(nc.semaphore("drain_sem"))
        pe_sem = ctx.enter_context(nc.semaphore("pe_sem"))
        block = ctx.enter_context(nc.Block())

        @block.sync
        def _(sync):
            sync.dma_start(id_t.ap(), id_d[:]).then_inc(id_sem, 16)
            for s in range(NSTEP):
                if s >= 1:
                    # DMA completions across steps are unordered; gate on
                    # the previous step's completions so cumulative
                    # thresholds imply the right data.
                    sync.wait_ge(load_sem, 16 * s)
                    sync.wait_ge(sm0_sem, 16 * s)
                    sync.wait_ge(sm1_sem, 16 * s)
                if s >= 2:
                    # tile reuse: step s-2 consumers must be done
                    sync.wait_ge(dve_sem, dve_at(s - 1, 0))
                    sync.wait_ge(pe_sem, 75 * (s - 1))
                b = s % 2
                sync.dma_start(xyz_t[b].ap(), xyz_d[s]).then_inc(load_sem, 16)
                sync.dma_start(sm_t[b][:, 0], sm_d[s, :, 0]).then_inc(
                    sm0_sem, 16)
                sync.dma_start(sm_t[b][:, 1], sm_d[s, :, 1]).then_inc(
                    sm1_sem, 16)
                if s >= 1:
                    sync.wait_ge(drain_sem, 3 * s)
                    sync.wait_ge(store_sem, 16 * (s - 1))
                    sync.dma_start(out_d[s - 1],
                                   out_t[(s - 1) % 2].ap()).then_inc(
                                       store_sem, 16)
            sync.wait_ge(drain_sem, 3 * NSTEP)
            sync.wait_ge(store_sem, 16 * (NSTEP - 1))
            sync.dma_start(out_d[NSTEP - 1],
                           out_t[(NSTEP - 1) % 2].ap()).then_inc(
                               store_sem, 16)

        @block.vector
        def _(vector):
            def subs(k, small=False):
                # diff[dy,i,dx,w] = xyz[e][dy,i,dx',w] - xyz[e0][2,i,2+w]
                # ISA allows 3 free dims -> one instruction per component i
                kh, ws = (1, 2) if small else (KH, WS)
                xt = xyz_t[k % 2]
                for i in range(3):
                    cen3 = bass.AP(xt, (2 * 3 + i) * WX + PAD,
                                   [[XP, 128], [0, kh], [0, 3], [1, ws]])
                    cen2 = bass.AP(xt, (2 * 3 + i) * WX + PAD,
                                   [[XP, 128], [0, kh], [0, 2], [1, ws]])
                    in_e = bass.AP(xt, i * WX,
                                   [[XP, 128], [3 * WX, kh], [2, 3], [1, ws]])
                    out_e = bass.AP(diff_t, i * KW * WS,
                                    [[DP, 128], [3 * KW * WS, kh],
                                     [2 * WS, 3], [1, ws]])
                    vector.tensor_tensor(out=out_e, in0=in_e, in1=cen3,
                                         op=SUB).then_inc(dve_sem)
                    in_o = bass.AP(xt, KH * 3 * WX + i * WX,
                                   [[XP, 128], [3 * WX, kh], [2, 2], [1, ws]])
                    out_o = bass.AP(diff_t, i * KW * WS + WS,
                                    [[DP, 128], [3 * KW * WS, kh],
                                     [2 * WS, 2], [1, ws]])
                    vector.tensor_tensor(out=out_o, in0=in_o, in1=cen2,
                                         op=SUB).then_inc(dve_sem)

            def d2adds(s, small=False):
                kh, kw, ws = (1, 1, 2) if small else (KH, KW, WS)
                d2 = bass.AP(d2_t[s % 2], 0,
                             [[GP, 128], [KW * WS, kh], [WS, kw], [1, ws]])
                sq_i = [bass.AP(sq_t, i * KW * WS,
                                [[DP, 128], [3 * KW * WS, kh], [WS, kw],
                                 [1, ws]]) for i in range(3)]
                vector.tensor_tensor(out=d2, in0=sq_i[0], in1=sq_i[1],
                                     op=ADD).then_inc(dve_sem)
                vector.tensor_tensor(out=d2, in0=d2, in1=sq_i[2],
                                     op=ADD).then_inc(dve_sem)

            def muls(s, dy):
                st, g5 = sm_t[s % 2], g5_t[s % 2]
                tt = t_t[(5 * s + dy) % 3]
                de = dxs_even(dy)
                stride = de[1] - de[0]
                out_e = bass.AP(tt, de[0] * C * WS,
                                [[TP, 128], [stride * C * WS, len(de)],
                                 [WS, C], [1, WS]])
                sm_e = bass.AP(st, dy * C * WX,
                               [[SP, 128], [stride, len(de)], [WX, C],
                                [1, WS]])
                g_e = bass.AP(g5, dy * KW * WS + de[0] * WS,
                              [[GP, 128], [stride * WS, len(de)], [0, C],
                               [1, WS]])
                vector.tensor_tensor(out=out_e, in0=sm_e, in1=g_e,
                                     op=MULT).then_inc(dve_sem)
                out_o = bass.AP(tt, C * WS,
                                [[TP, 128], [2 * C * WS, 2], [WS, C],
                                 [1, WS]])
                sm_o = bass.AP(st, SE + dy * C * WX,
                               [[SP, 128], [2, 2], [WX, C], [1, WS]])
                g_o = bass.AP(g5, dy * KW * WS + WS,
                              [[GP, 128], [2 * WS, 2], [0, C], [1, WS]])
                vector.tensor_tensor(out=out_o, in0=sm_o, in1=g_o,
                                     op=MULT).then_inc(dve_sem)

            vector.wait_ge(load_sem, 16)
            subs(0)
            vector.wait_ge(act_sem, 1)
            d2adds(0)
            for s in range(NSTEP):
                for dy in range(KH):
                    # t buffer round-robin: block b=5s+dy reuses t[b%3],
                    # free once PE finished block b-3
                    pe_need = 15 * (5 * s + dy - 2)
                    if pe_need > 0:
                        vector.wait_ge(pe_sem, pe_need)
                    if dy == 0:
                        vector.wait_ge(act_sem, 2 * s + 2)
                        vector.wait_ge(sm0_sem, 16 * (s + 1))
                        vector.wait_ge(sm1_sem, 16 * (s + 1))
                    muls(s, dy)
                    if dy == 1:
                        if s + 1 < NSTEP:
                            vector.wait_ge(load_sem, 16 * (s + 2))
                            subs(s + 1)
                        else:
                            subs(s, small=True)
                    elif dy == 3:
                        if s + 1 < NSTEP:
                            vector.wait_ge(act_sem, 2 * s + 3)
                            d2adds(s + 1)
                        else:
                            d2adds(s, small=True)

        @block.scalar
        def _(scalar):
            EXP = mybir.ActivationFunctionType.Exp
            SQR = mybir.ActivationFunctionType.Square

            def sq(s):
                scalar.wait_ge(dve_sem, dve_at(s - 1, 10) if s else 6)
                scalar.activation(
                    out=bass.AP(sq_t, 0, [[DP, 128], [1, DP]]),
                    in_=bass.AP(diff_t, 0, [[DP, 128], [1, DP]]),
                    func=SQR).then_inc(act_sem)

            def exp(s):
                scalar.wait_ge(dve_sem,
                               dve_at(s - 1, 16) if s else DVE_PRO)
                scalar.activation(
                    out=bass.AP(g5_t[s % 2], 0, [[GP, 128], [1, GP]]),
                    in_=bass.AP(d2_t[s % 2], 0, [[GP, 128], [1, GP]]),
                    func=EXP, scale=-0.5).then_inc(act_sem)

            sq(0)
            exp(0)
            for s in range(NSTEP):
                if s + 1 < NSTEP:
                    sq(s + 1)
                    exp(s + 1)
                scalar.wait_ge(pe_sem, 75 * (s + 1))
                if s >= 2:
                    scalar.wait_ge(store_sem, 16 * (s - 1))
                for lo, ln in CT:
                    scalar.activation(
                        out=bass.AP(out_t[s % 2], lo, [[OP, 128], [1, ln]]),
                        in_=ps_t[s % 2].ap()[:, lo:lo + ln],
                        func=mybir.ActivationFunctionType.Copy).then_inc(
                            drain_sem)

        @block.tensor
        def _(tensor):
            tensor.wait_ge(id_sem, 16)
            lhsT = id_t.ap()
            for s in range(NSTEP):
                ps = ps_t[s % 2]
                for dy in range(KH):
                    tensor.wait_ge(dve_sem, dve_at(s, (2, 4, 12, 14, 18)[dy]))
                    if dy == 0 and s >= 1:
                        tensor.wait_ge(drain_sem, 3 * (s - 1))
                    if dy == 2:
                        tensor.wait_ge(sm0_sem, 16 * (s + 1))
                    slots = (0, 1, 3, 4) if dy == 2 else range(KW)
                    for dxs in slots:
                        for lo, ln in CT:
                            rhs = bass.AP(t_t[(5 * s + dy) % 3],
                                          dxs * C * WS + lo,
                                          [[TP, 128], [1, ln]])
                            tensor.matmul(
                                ps.ap()[:, lo:lo + ln], lhsT, rhs,
                                start=(dy == 0 and dxs == 0),
                                stop=(dy == 4 and dxs == 4),
                                skip_group_check=True).then_inc(pe_sem)
                    if dy == 2:
                        # center term: g == 1, accumulate sm directly
                        for lo, ln in CT:
                            c0, ncl = lo // WS, ln // WS
                            rhs = bass.AP(sm_t[s % 2],
                                          2 * C * WX + c0 * WX + PAD,
                                          [[SP, 128], [WX, ncl], [1, WS]])
                            tensor.matmul(
                                ps.ap()[:, lo:lo + ln], lhsT, rhs,
                                start=False, stop=False,
                                skip_group_check=True).then_inc(pe_sem)

    return nc


def _prep_core(xyz, softmax, mask, core):
    """Host-side slab bake (bf16): fold mask into softmax, pad, and lay
    out dy-shifted parity-duplicated windows per partition row."""
    n, half = core // 2, core % 2
    w0 = WCORE * half
    wp_sz = WCORE + 2 * PAD + 1
    lo, hi = w0 - PAD, w0 + WCORE + PAD + 1
    glo, ghi = max(lo, 0), min(hi, W)

    sm_m = (softmax[n][:, :, glo:ghi] *
            mask[n][None, :, glo:ghi]).astype(BF16)
    smp = np.zeros((HH, C, wp_sz), BF16)
    smp[PAD:PAD + H, :, glo - lo:ghi - lo] = sm_m.transpose(1, 0, 2)
    xyzp = np.zeros((HH, 3, wp_sz), BF16)
    xyzp[PAD:PAD + H, :, glo - lo:ghi - lo] = (
        xyz[n][:, :, glo:ghi].transpose(1, 0, 2).astype(BF16))

    sm5 = np.empty((NSTEP, 128, 2, KH, C, WX), BF16)
    xyz5 = np.empty((NSTEP, 128, 2, KH, 3, WX), BF16)
    for s in range(NSTEP):
        for chunk in range(2):
            wb = WS * (s + NSTEP * chunk)
            pr = slice(64 * chunk, 64 * chunk + 64)
            for dy in range(KH):
                for e in range(2):
                    sm5[s, pr, e, dy] = smp[dy:dy + 64, :, wb + e:wb + e + WX]
                    xyz5[s, pr, e, dy] = xyzp[dy:dy + 64, :,
                                              wb + e:wb + e + WX]
    ident = np.eye(128, dtype=BF16)
    return {"sm_in": sm5, "xyz_in": xyz5, "ident_in": ident}


def make_in_maps(xyz, softmax, mask):
    return [_prep_core(xyz, softmax, mask, k) for k in range(8)]


def assemble_out(results):
    out = np.empty((N, C, H, W), np.float32)
    for core in range(8):
        n, half = core // 2, core % 2
        w0 = WCORE * half
        o = np.asarray(results[core]["out_d"]).astype(np.float32)
        # [s, chunk*64+h, c, j] -> [c, h, chunk*512 + s*64 + j]
        o = o.reshape(NSTEP, 2, H, C, WS)
        out[n, :, :, w0:w0 + WCORE] = o.transpose(3, 2, 1, 0, 4).reshape(
            C, H, WCORE)
    return out


def get_nc():
    if "nc" not in _CACHE:
        _CACHE["nc"] = _build_nc()
    return _CACHE["nc"]


def kernel(xyz, softmax, mask, trace=False, trace_kwargs=None):
    nc = get_nc()
    in_maps = make_in_maps(np.asarray(xyz), np.asarray(softmax),
                           np.asarray(mask))
    res = run_bass_kernel_spmd(nc, in_maps, list(range(8)), trace=trace,
                               **(trace_kwargs or {}))
    out = assemble_out(res.results)
    if trace:
        return out, res
    return out


# revision 10
# speedup vs baseline: 1.0441x; 1.0441x over previous
"""LocallyConnectedXYZLayer Trainium2 kernel (v2).

out[n,c,h,w] = sum_{dy,dx in 5x5} exp(-|xyz(n,:,h+dy-2,w+dx-2)-xyz(n,:,h,w)|^2/2)
               * (softmax*mask)(n,c,h+dy-2,w+dx-2)        (zero-padded)

Sharding: 8 cores = (batch n = core//2) x (W half = core%2).
Per-core layout: partitions = 2 w-chunks x 64 h rows; free dims carry
(dy, dx, c, w).  dy window shifts are host-baked into per-partition rows;
dx shifts are free-dim slices, duplicated at +0/+1 (parity e) so every
strided slice stays 4-byte aligned and the DVE runs tensor_tensor in
2x bf16 mode throughout.

Engine split per step (8 steps of 64 interior w per chunk):
  DVE   : diff = xyz_shift - xyz_center (2 merged subs), d2 = sum of
          squares (2 adds), t = g * sm (10 merged muls, c broadcast via
          stride-0 dim).
  ACT   : Square(diff), Exp(-d2/2), PSUM->SBUF output drains (all three
          live in the exp_and_others table set -> one table load).
  PE    : accumulates the 25 shifted products into PSUM with
          matmul(Identity, t_k) -- the adds cost nothing on DVE; the
          center term (g==1) is accumulated straight from sm, skipping
          its multiply.  fp32 PSUM accumulation, bf16 output store.
mask is folded into softmax on the host.
"""

import sys
from contextlib import ExitStack

import numpy as np

sys.path.insert(0, "/opt/trn_rl_repo")

import ml_dtypes  # noqa: E402

import concourse.bass as bass  # noqa: E402
from concourse import mybir  # noqa: E402
from concourse.bass_utils import run_bass_kernel_spmd  # noqa: E402

BF16 = ml_dtypes.bfloat16

N, C, H, W = 4, 20, 64, 2048
KH = KW = 5
PAD = 2
HH = H + 2 * PAD  # 68 padded rows
WCORE = W // 2  # 1024 interior w per core
NSTEP = 8
WS = WCORE // (2 * NSTEP)  # 64 interior w per (step, chunk)
WX = WS + 2 * PAD  # 68 w extent (halo 2 each side)

# per-partition element pitches
XP = 2 * KH * 3 * WX          # xyz tile pitch (2040)
SP = 2 * KH * C * WX          # sm tile pitch (27200)
SE = KH * C * WX              # sm parity block (6800)
DP = KH * 3 * KW * WS         # diff pitch (4800)
GP = KH * KW * WS             # d2/g pitch (1600)
TP = KW * C * WS              # t pitch (6400)
OP = C * WS                   # out pitch (1280)

_CACHE = {}


def _build_nc():
    """Raw-Bass program; cross-engine sync is standalone wait_ge
    instructions plus one then_inc per producer op (walrus allows at most
    one sync command per instruction)."""
    nc = bass.Bass("TRN2", target_bir_lowering=False, debug=False)
    bf = mybir.dt.bfloat16
    f32 = mybir.dt.float32
    sm_d = nc.dram_tensor("sm_in", [NSTEP, 128, 2, KH, C, WX], bf,
                          kind="ExternalInput")
    xyz_d = nc.dram_tensor("xyz_in", [NSTEP, 128, 2, KH, 3, WX], bf,
                           kind="ExternalInput")
    id_d = nc.dram_tensor("ident_in", [128, 128], bf, kind="ExternalInput")
    out_d = nc.dram_tensor("out_d", [NSTEP, 128, C, WS], bf,
                           kind="ExternalOutput")

    def sb(name, shape, dt):
        return nc.alloc_sbuf_tensor(name, list(shape), dt)

    xyz_t = [sb(f"xyz{i}", [128, 2, KH, 3, WX], bf) for i in range(2)]
    sm_t = [sb(f"sm{i}", [128, 2, KH, C, WX], bf) for i in range(2)]
    diff_t = sb("diff", [128, KH, 3, KW, WS], bf)
    sq_t = sb("sq", [128, KH, 3, KW, WS], bf)
    d2_t = [sb(f"d2_{i}", [128, KH, KW, WS], bf) for i in range(2)]
    g5_t = [sb(f"g5_{i}", [128, KH, KW, WS], bf) for i in range(2)]
    t_t = [sb(f"t{i}", [128, KW, C, WS], bf) for i in range(3)]
    out_t = [sb(f"out{i}", [128, C, WS], bf) for i in range(2)]
    id_t = sb("ident", [128, 128], bf)
    ps_t = [nc.alloc_psum_tensor(f"ps{i}", [128, OP], f32) for i in range(2)]

    ADD, MULT, SUB = (mybir.AluOpType.add, mybir.AluOpType.mult,
                      mybir.AluOpType.subtract)

    # column tiles for the PE/PSUM accumulation (c,w flattened)
    CT = [(0, 512), (512, 512), (1024, 256)]
    # dx slots per (dy, parity): even slots from parity-0 data, odd from
    # parity-1; dy==2 drops the center (dx==2) from the even list.
    def dxs_even(dy):
        return (0, 4) if dy == 2 else (0, 2, 4)

    # DVE ops per step block: 10 muls, 6 subs + 2 d2adds (for s+1).
    # Block positions: muls dy0 1-2, dy1 3-4, subs 5-10, dy2 11-12,
    # dy3 13-14, d2a 15-16, dy4 17-18 -- exp(s+1) overlaps the dy4
    # muls instead of stalling the next step.  Prologue: 6 subs + 2 d2a.
    DVE_STEP = 18
    DVE_PRO = 8

    def dve_at(s, pos):
        # semaphore value after `pos` ops of step-s block
        return DVE_PRO + DVE_STEP * s + pos

    with ExitStack() as ctx:
        load_sem = ctx.enter_context(nc.semaphore("load_sem"))
        sm0_sem = ctx.enter_context(nc.semaphore("sm0_sem"))
        sm1_sem = ctx.enter_context(nc.semaphore("sm1_sem"))
        id_sem = ctx.enter_context(nc.semaphore("id_sem"))
        store_sem = ctx.enter_context(nc.semaphore("store_sem"))
        dve_sem = ctx.enter_context(nc.semaphore("dve_sem"))
        act_sem = ctx.enter_context(nc.semaphore("act_sem"))
        drain_sem = ctx.enter_context(nc.semaphore("drain_sem"))
        pe_sem = ctx.enter_context(nc.semaphore("pe_sem"))
        block = ctx.enter_context(nc.Block())

        @block.sync
        def _(sync):
            sync.dma_start(id_t.ap(), id_d[:]).then_inc(id_sem, 16)
            for s in range(NSTEP):
                if s >= 1:
                    # DMA completions across steps are unordered; gate on
                    # the previous step's completions so cumulative
                    # thresholds imply the right data.
                    sync.wait_ge(load_sem, 16 * s)
                    sync.wait_ge(sm0_sem, 16 * s)
                if s >= 2:
                    # tile reuse: step s-2 consumers must be done
                    sync.wait_ge(dve_sem, dve_at(s - 1, 0))
                    sync.wait_ge(pe_sem, 75 * (s - 1))
                b = s % 2
                sync.dma_start(xyz_t[b].ap(), xyz_d[s]).then_inc(load_sem, 16)
                sync.dma_start(sm_t[b][:, 0], sm_d[s, :, 0]).then_inc(
                    sm0_sem, 16)

        @block.gpsimd
        def _(gp):
            # second DMA queue: sm parity-1 loads + output stores
            for s in range(NSTEP):
                if s >= 1:
                    gp.wait_ge(sm1_sem, 16 * s)
                if s >= 2:
                    gp.wait_ge(dve_sem, dve_at(s - 1, 0))
                    gp.wait_ge(pe_sem, 75 * (s - 1))
                b = s % 2
                gp.dma_start(sm_t[b][:, 1], sm_d[s, :, 1]).then_inc(
                    sm1_sem, 16)
                if s >= 1:
                    gp.wait_ge(drain_sem, 3 * s)
                    gp.wait_ge(store_sem, 16 * (s - 1))
                    gp.dma_start(out_d[s - 1],
                                 out_t[(s - 1) % 2].ap()).then_inc(
                                     store_sem, 16)
            gp.wait_ge(drain_sem, 3 * NSTEP)
            gp.wait_ge(store_sem, 16 * (NSTEP - 1))
            gp.dma_start(out_d[NSTEP - 1],
                         out_t[(NSTEP - 1) % 2].ap()).then_inc(
                             store_sem, 16)

        @block.vector
        def _(vector):
            def subs(k, small=False):
                # diff[dy,i,dx,w] = xyz[e][dy,i,dx',w] - xyz[e0][2,i,2+w]
                # ISA allows 3 free dims -> one instruction per component i
                kh, ws = (1, 2) if small else (KH, WS)
                xt = xyz_t[k % 2]
                for i in range(3):
                    cen3 = bass.AP(xt, (2 * 3 + i) * WX + PAD,
                                   [[XP, 128], [0, kh], [0, 3], [1, ws]])
                    cen2 = bass.AP(xt, (2 * 3 + i) * WX + PAD,
                                   [[XP, 128], [0, kh], [0, 2], [1, ws]])
                    in_e = bass.AP(xt, i * WX,
                                   [[XP, 128], [3 * WX, kh], [2, 3], [1, ws]])
                    out_e = bass.AP(diff_t, i * KW * WS,
                                    [[DP, 128], [3 * KW * WS, kh],
                                     [2 * WS, 3], [1, ws]])
                    vector.tensor_tensor(out=out_e, in0=in_e, in1=cen3,
                                         op=SUB).then_inc(dve_sem)
                    in_o = bass.AP(xt, KH * 3 * WX + i * WX,
                                   [[XP, 128], [3 * WX, kh], [2, 2], [1, ws]])
                    out_o = bass.AP(diff_t, i * KW * WS + WS,
                                    [[DP, 128], [3 * KW * WS, kh],
                                     [2 * WS, 2], [1, ws]])
                    vector.tensor_tensor(out=out_o, in0=in_o, in1=cen2,
                                         op=SUB).then_inc(dve_sem)

            def d2adds(s, small=False):
                kh, kw, ws = (1, 1, 2) if small else (KH, KW, WS)
                d2 = bass.AP(d2_t[s % 2], 0,
                             [[GP, 128], [KW * WS, kh], [WS, kw], [1, ws]])
                sq_i = [bass.AP(sq_t, i * KW * WS,
                                [[DP, 128], [3 * KW * WS, kh], [WS, kw],
                                 [1, ws]]) for i in range(3)]
                vector.tensor_tensor(out=d2, in0=sq_i[0], in1=sq_i[1],
                                     op=ADD).then_inc(dve_sem)
                vector.tensor_tensor(out=d2, in0=d2, in1=sq_i[2],
                                     op=ADD).then_inc(dve_sem)

            def muls(s, dy):
                st, g5 = sm_t[s % 2], g5_t[s % 2]
                tt = t_t[(5 * s + dy) % 3]
                de = dxs_even(dy)
                stride = de[1] - de[0]
                out_e = bass.AP(tt, de[0] * C * WS,
                                [[TP, 128], [stride * C * WS, len(de)],
                                 [WS, C], [1, WS]])
                sm_e = bass.AP(st, dy * C * WX,
                               [[SP, 128], [stride, len(de)], [WX, C],
                                [1, WS]])
                g_e = bass.AP(g5, dy * KW * WS + de[0] * WS,
                              [[GP, 128], [stride * WS, len(de)], [0, C],
                               [1, WS]])
                vector.tensor_tensor(out=out_e, in0=sm_e, in1=g_e,
                                     op=MULT).then_inc(dve_sem)
                out_o = bass.AP(tt, C * WS,
                                [[TP, 128], [2 * C * WS, 2], [WS, C],
                                 [1, WS]])
                sm_o = bass.AP(st, SE + dy * C * WX,
                               [[SP, 128], [2, 2], [WX, C], [1, WS]])
                g_o = bass.AP(g5, dy * KW * WS + WS,
                              [[GP, 128], [2 * WS, 2], [0, C], [1, WS]])
                vector.tensor_tensor(out=out_o, in0=sm_o, in1=g_o,
                                     op=MULT).then_inc(dve_sem)

            vector.wait_ge(load_sem, 16)
            subs(0)
            vector.wait_ge(act_sem, 1)
            d2adds(0)
            for s in range(NSTEP):
                for dy in range(KH):
                    # t buffer round-robin: block b=5s+dy reuses t[b%3],
                    # free once PE finished block b-3
                    pe_need = 15 * (5 * s + dy - 2)
                    if pe_need > 0:
                        vector.wait_ge(pe_sem, pe_need)
                    if dy == 0:
                        vector.wait_ge(act_sem, 2 * s + 2)
                        vector.wait_ge(sm0_sem, 16 * (s + 1))
                        vector.wait_ge(sm1_sem, 16 * (s + 1))
                    muls(s, dy)
                    if dy == 1:
                        if s + 1 < NSTEP:
                            vector.wait_ge(load_sem, 16 * (s + 2))
                            subs(s + 1)
                        else:
                            subs(s, small=True)
                    elif dy == 3:
                        if s + 1 < NSTEP:
                            vector.wait_ge(act_sem, 2 * s + 3)
                            d2adds(s + 1)
                        else:
                            d2adds(s, small=True)

        @block.scalar
        def _(scalar):
            EXP = mybir.ActivationFunctionType.Exp
            SQR = mybir.ActivationFunctionType.Square

            def sq(s):
                scalar.wait_ge(dve_sem, dve_at(s - 1, 10) if s else 6)
                scalar.activation(
                    out=bass.AP(sq_t, 0, [[DP, 128], [1, DP]]),
                    in_=bass.AP(diff_t, 0, [[DP, 128], [1, DP]]),
                    func=SQR).then_inc(act_sem)

            def exp(s):
                scalar.wait_ge(dve_sem,
                               dve_at(s - 1, 16) if s else DVE_PRO)
                scalar.activation(
                    out=bass.AP(g5_t[s % 2], 0, [[GP, 128], [1, GP]]),
                    in_=bass.AP(d2_t[s % 2], 0, [[GP, 128], [1, GP]]),
                    func=EXP, scale=-0.5).then_inc(act_sem)

            sq(0)
            exp(0)
            for s in range(NSTEP):
                if s + 1 < NSTEP:
                    sq(s + 1)
                    exp(s + 1)
                scalar.wait_ge(pe_sem, 75 * (s + 1))
                if s >= 2:
                    scalar.wait_ge(store_sem, 16 * (s - 1))
                for lo, ln in CT:
                    scalar.activation(
                        out=bass.AP(out_t[s % 2], lo, [[OP, 128], [1, ln]]),
                        in_=ps_t[s % 2].ap()[:, lo:lo + ln],
                        func=mybir.ActivationFunctionType.Copy).then_inc(
                            drain_sem)

        @block.tensor
        def _(tensor):
            tensor.wait_ge(id_sem, 16)
            lhsT = id_t.ap()
            for s in range(NSTEP):
                ps = ps_t[s % 2]
                for dy in range(KH):
                    tensor.wait_ge(dve_sem, dve_at(s, (2, 4, 12, 14, 18)[dy]))
                    if dy == 0 and s >= 1:
                        tensor.wait_ge(drain_sem, 3 * (s - 1))
                    if dy == 2:
                        tensor.wait_ge(sm0_sem, 16 * (s + 1))
                    slots = (0, 1, 3, 4) if dy == 2 else range(KW)
                    for dxs in slots:
                        for lo, ln in CT:
                            rhs = bass.AP(t_t[(5 * s + dy) % 3],
                                          dxs * C * WS + lo,
                                          [[TP, 128], [1, ln]])
                            tensor.matmul(
                                ps.ap()[:, lo:lo + ln], lhsT, rhs,
                                start=(dy == 0 and dxs == 0),
                                stop=(dy == 4 and dxs == 4),
                                skip_group_check=True).then_inc(pe_sem)
                    if dy == 2:
                        # center term: g == 1, accumulate sm directly
                        for lo, ln in CT:
                            c0, ncl = lo // WS, ln // WS
                            rhs = bass.AP(sm_t[s % 2],
                                          2 * C * WX + c0 * WX + PAD,
                                          [[SP, 128], [WX, ncl], [1, WS]])
                            tensor.matmul(
                                ps.ap()[:, lo:lo + ln], lhsT, rhs,
                                start=False, stop=False,
                                skip_group_check=True).then_inc(pe_sem)

    return nc


def _prep_core(xyz, softmax, mask, core):
    """Host-side slab bake (bf16): fold mask into softmax, pad, and lay
    out dy-shifted parity-duplicated windows per partition row."""
    n, half = core // 2, core % 2
    w0 = WCORE * half
    wp_sz = WCORE + 2 * PAD + 1
    lo, hi = w0 - PAD, w0 + WCORE + PAD + 1
    glo, ghi = max(lo, 0), min(hi, W)

    sm_m = (softmax[n][:, :, glo:ghi] *
            mask[n][None, :, glo:ghi]).astype(BF16)
    smp = np.zeros((HH, C, wp_sz), BF16)
    smp[PAD:PAD + H, :, glo - lo:ghi - lo] = sm_m.transpose(1, 0, 2)
    xyzp = np.zeros((HH, 3, wp_sz), BF16)
    xyzp[PAD:PAD + H, :, glo - lo:ghi - lo] = (
        xyz[n][:, :, glo:ghi].transpose(1, 0, 2).astype(BF16))

    sm5 = np.empty((NSTEP, 128, 2, KH, C, WX), BF16)
    xyz5 = np.empty((NSTEP, 128, 2, KH, 3, WX), BF16)
    for s in range(NSTEP):
        for chunk in range(2):
            wb = WS * (s + NSTEP * chunk)
            pr = slice(64 * chunk, 64 * chunk + 64)
            for dy in range(KH):
                for e in range(2):
                    sm5[s, pr, e, dy] = smp[dy:dy + 64, :, wb + e:wb + e + WX]
                    xyz5[s, pr, e, dy] = xyzp[dy:dy + 64, :,
                                              wb + e:wb + e + WX]
    ident = np.eye(128, dtype=BF16)
    return {"sm_in": sm5, "xyz_in": xyz5, "ident_in": ident}


def make_in_maps(xyz, softmax, mask):
    return [_prep_core(xyz, softmax, mask, k) for k in range(8)]


def assemble_out(results):
    out = np.empty((N, C, H, W), np.float32)
    for core in range(8):
        n, half = core // 2, core % 2
        w0 = WCORE * half
        o = np.asarray(results[core]["out_d"]).astype(np.float32)
        # [s, chunk*64+h, c, j] -> [c, h, chunk*512 + s*64 + j]
        o = o.reshape(NSTEP, 2, H, C, WS)
        out[n, :, :, w0:w0 + WCORE] = o.transpose(3, 2, 1, 0, 4).reshape(
            C, H, WCORE)
    return out


def get_nc():
    if "nc" not in _CACHE:
        _CACHE["nc"] = _build_nc()
    return _CACHE["nc"]


def kernel(xyz, softmax, mask, trace=False, trace_kwargs=None):
    nc = get_nc()
    in_maps = make_in_maps(np.asarray(xyz), np.asarray(softmax),
                           np.asarray(mask))
    res = run_bass_kernel_spmd(nc, in_maps, list(range(8)), trace=trace,
                               **(trace_kwargs or {}))
    out = assemble_out(res.results)
    if trace:
        return out, res
    return out


# revision 31
# speedup vs baseline: 1.1263x; 1.0787x over previous
"""LocallyConnectedXYZLayer Trainium2 kernel (v4).

out[n,c,h,w] = sum_{dy,dx in 5x5} exp(-|xyz(n,:,h+dy-2,w+dx-2)-xyz(n,:,h,w)|^2/2)
               * (softmax*mask)(n,c,h+dy-2,w+dx-2)        (zero-padded)

Sharding: 8 cores = (batch n = core//2) x (W half = core%2).
Per-core layout: partitions = 2 w-chunks x 64 h rows; free dims carry
(dy, dx, c, w).  dy window shifts are host-baked into per-partition rows;
dx shifts are free-dim slices.  Every strided slice keeps a 4-byte-aligned
base by splitting work into even-dx and odd-dx instructions, where odd-dx
reads a +1-shifted parity copy of the data -- so the DVE runs
tensor_tensor in 2x bf16 mode throughout.

Engine split per step (8 steps of 64 interior w per chunk):
  DVE   : diff = xyz_shift - xyz_center (6 subs), d2 = sum of squares
          (2 adds), t = g * sm (10 muls, c broadcast via stride-0 dim).
  ACT   : Square(diff), Exp(-d2/2), the sm odd-parity shift copies
          (one per dy), PSUM->SBUF output drains (all in the
          exp_and_others table set -> one table load).
  PE    : accumulates the 25 shifted products into PSUM with
          matmul(Identity, t_k); the center term (g==1) is accumulated
          straight from sm, skipping its multiply.  fp32 accumulation,
          bf16 output store.
  DMA   : sm arrives single-parity in per-dy chunks on the sync queue
          (the +1 parity copy is built on-device by ACT); xyz (small)
          ships both parities; stores ride the gpsimd queue.
mask is folded into softmax on the host.
"""

import sys
from contextlib import ExitStack

import numpy as np

sys.path.insert(0, "/opt/trn_rl_repo")

import ml_dtypes  # noqa: E402

import concourse.bass as bass  # noqa: E402
from concourse import mybir  # noqa: E402
from concourse.bass_utils import run_bass_kernel_spmd  # noqa: E402

BF16 = ml_dtypes.bfloat16

N, C, H, W = 4, 20, 64, 2048
KH = KW = 5
PAD = 2
HH = H + 2 * PAD  # 68 padded rows
WCORE = W // 2  # 1024 interior w per core
NSTEP = 8
WS = WCORE // (2 * NSTEP)  # 64 interior w per (step, chunk)
WX = WS + 2 * PAD  # 68 w extent (halo 2 each side)

# per-partition element pitches
XP = 2 * KH * 3 * WX          # xyz tile pitch (2040)
SP = 2 * KH * C * WX          # sm tile pitch (27200)
SE = KH * C * WX              # sm parity block (6800)
DP = KH * 3 * KW * WS         # diff pitch (4800)
GP = KH * KW * WS             # d2/g pitch (1600)
TP = KW * C * WS              # t pitch (6400)
OP = C * WS                   # out pitch (1280)

_CACHE = {}


def _build_nc():
    """Raw-Bass program; cross-engine sync is standalone wait_ge
    instructions plus one then_inc per producer op (walrus allows at most
    one sync command per instruction)."""
    nc = bass.Bass("TRN2", target_bir_lowering=False, debug=False)
    bf = mybir.dt.bfloat16
    f32 = mybir.dt.float32
    sm_d = nc.dram_tensor("sm_in", [NSTEP, 128, KH, C, WX], bf,
                          kind="ExternalInput")
    xyz_d = nc.dram_tensor("xyz_in", [NSTEP, 128, 2, KH, 3, WX], bf,
                           kind="ExternalInput")
    id_d = nc.dram_tensor("ident_in", [128, 128], bf, kind="ExternalInput")
    out_d = nc.dram_tensor("out_d", [NSTEP, 128, C, WS], bf,
                           kind="ExternalOutput")

    def sb(name, shape, dt):
        return nc.alloc_sbuf_tensor(name, list(shape), dt)

    xyz_t = [sb(f"xyz{i}", [128, 2, KH, 3, WX], bf) for i in range(2)]
    sm_t = [sb(f"sm{i}", [128, 2, KH, C, WX], bf) for i in range(2)]
    diff_t = sb("diff", [128, KH, 3, KW, WS], bf)
    sq_t = sb("sq", [128, KH, 3, KW, WS], bf)
    d2_t = [sb(f"d2_{i}", [128, KH, KW, WS], bf) for i in range(2)]
    g5_t = [sb(f"g5_{i}", [128, KH, KW, WS], bf) for i in range(2)]
    t_t = [sb(f"t{i}", [128, KW, C, WS], bf) for i in range(3)]
    out_t = [sb(f"out{i}", [128, C, WS], bf) for i in range(2)]
    id_t = sb("ident", [128, 128], bf)
    ps_t = [nc.alloc_psum_tensor(f"ps{i}", [128, OP], f32) for i in range(2)]

    ADD, MULT, SUB = (mybir.AluOpType.add, mybir.AluOpType.mult,
                      mybir.AluOpType.subtract)

    # column tiles for the PE/PSUM accumulation (c,w flattened)
    CT = [(0, 512), (512, 512), (1024, 256)]

    def dxs_even(dy):
        return (0, 4) if dy == 2 else (0, 2, 4)

    # DVE ops per step block: 10 muls, 6 subs + 2 d2adds (for s+1).
    # Block positions: muls dy0 1-2, dy1 3-4, subs 5-10, dy2 11-12,
    # dy3 13-14, d2a 15-16, dy4 17-18 -- exp(s+1) overlaps the dy4
    # muls instead of stalling the next step.  Prologue: 6 subs + 2 d2a.
    DVE_STEP = 18
    POS_EVEN = (1, 3, 11, 13, 17)   # even-parity mul position per dy
    POS_ODD = (2, 4, 12, 14, 18)
    # Step 0 pipelines its d2 chain per-dy (dy0 first, then dy1-4), so
    # block 0 holds 20 ops after a 14-op prologue; this LUT maps the
    # standard block positions onto block 0's actual op numbers.
    S0 = {1: 15, 2: 16, 3: 19, 4: 20, 10: 26, 11: 27, 12: 28, 13: 29,
          14: 30, 16: 32, 17: 33, 18: 34}

    def dve_at(s, pos):
        if s == 0:
            return S0[pos]
        return 16 + DVE_STEP * s + pos

    # ACT ops per step: sq / exp / pcs_dy0..4 (7), plus 3 drains on
    # drain_sem.  pcs_dy2..4 are emitted after the drains so they fill
    # the ACT idle window at the start of the next step.
    # ACT prologue: sq_dy0=1, exp_dy0=2, sq_dy14=3, exp_dy14=4,
    # pcs(0,k)=5+k; steady state keeps 7 ops/step.
    def act_sq(s):
        return 7 * s + 3

    def act_exp(s, dy=4):
        if s == 0:
            return 2 if dy == 0 else 6
        return 7 * s + 4

    def act_pcs(s, k):
        if s == 0:
            return (3, 5, 7, 8, 9)[k]
        return 7 * s + 5 + k

    # sm0_sem arrival thresholds; step 0's dy0-1 piece rides the gpsimd
    # queue with its own sma_sem, so sm0_sem counts one DMA per step
    def smc(s, dy):
        return 16 * (s + 1)

    with ExitStack() as ctx:
        load_sem = ctx.enter_context(nc.semaphore("load_sem"))
        sm0_sem = ctx.enter_context(nc.semaphore("sm0_sem"))
        sma_sem = ctx.enter_context(nc.semaphore("sma_sem"))
        id_sem = ctx.enter_context(nc.semaphore("id_sem"))
        store_sem = ctx.enter_context(nc.semaphore("store_sem"))
        dve_sem = ctx.enter_context(nc.semaphore("dve_sem"))
        act_sem = ctx.enter_context(nc.semaphore("act_sem"))
        drain_sem = ctx.enter_context(nc.semaphore("drain_sem"))
        pe_sem = ctx.enter_context(nc.semaphore("pe_sem"))
        block = ctx.enter_context(nc.Block())

        @block.sync
        def _(sync):
            for s in range(NSTEP):
                if s >= 1:
                    # DMA completions across steps are unordered; gate on
                    # the previous step's completions so cumulative
                    # thresholds imply the right data.
                    sync.wait_ge(load_sem, 16 * s)
                    sync.wait_ge(sm0_sem, smc(s - 1, 4))
                if s >= 2:
                    # tile reuse: step s-2 consumers must be done
                    sync.wait_ge(dve_sem, dve_at(s - 1, 0))
                    sync.wait_ge(pe_sem, 75 * (s - 1))
                b = s % 2
                sync.dma_start(xyz_t[b].ap(), xyz_d[s]).then_inc(load_sem, 16)
                if s == 0:
                    # dy0-1 piece ships on the gpsimd queue in parallel
                    sync.dma_start(sm_t[b][:, 0, 2:],
                                   sm_d[s, :, 2:]).then_inc(sm0_sem, 16)
                else:
                    sync.dma_start(sm_t[b][:, 0],
                                   sm_d[s]).then_inc(sm0_sem, 16)

        @block.gpsimd
        def _(gp):
            # second DMA queue: step-0 sm dy0-1 piece + identity load +
            # output stores (last step split per column tile so the store
            # overlaps the final drains)
            gp.dma_start(sm_t[0][:, 0, 0:2],
                         sm_d[0, :, 0:2]).then_inc(sma_sem, 16)
            gp.dma_start(id_t.ap(), id_d[:]).then_inc(id_sem, 16)
            for s in range(NSTEP):
                if s < NSTEP - 1:
                    gp.wait_ge(drain_sem, 3 * (s + 1))
                    if s >= 1:
                        gp.wait_ge(store_sem, 16 * s)
                    gp.dma_start(out_d[s], out_t[s % 2].ap()).then_inc(
                        store_sem, 16)
                else:
                    gp.wait_ge(store_sem, 16 * s)
                    for k, (lo, ln) in enumerate(CT):
                        c0, c1 = lo // WS, (lo + ln) // WS
                        gp.wait_ge(drain_sem, 3 * s + k + 1)
                        gp.dma_start(
                            out_d[s, :, c0:c1],
                            bass.AP(out_t[s % 2], lo,
                                    [[OP, 128], [1, ln]])).then_inc(
                                        store_sem, 16)

        @block.vector
        def _(vector):
            def subs(k, lo=0, hi=KH, small=False):
                # diff[dy,i,dx,w] = xyz[e][dy,i,dx',w] - xyz[e0][2,i,2+w]
                # ISA allows 3 free dims -> one instruction per component i
                kh, ws = (1, 2) if small else (hi - lo, WS)
                xt = xyz_t[k % 2]
                for i in range(3):
                    cen3 = bass.AP(xt, (2 * 3 + i) * WX + PAD,
                                   [[XP, 128], [0, kh], [0, 3], [1, ws]])
                    cen2 = bass.AP(xt, (2 * 3 + i) * WX + PAD,
                                   [[XP, 128], [0, kh], [0, 2], [1, ws]])
                    in_e = bass.AP(xt, lo * 3 * WX + i * WX,
                                   [[XP, 128], [3 * WX, kh], [2, 3], [1, ws]])
                    out_e = bass.AP(diff_t, lo * 3 * KW * WS + i * KW * WS,
                                    [[DP, 128], [3 * KW * WS, kh],
                                     [2 * WS, 3], [1, ws]])
                    vector.tensor_tensor(out=out_e, in0=in_e, in1=cen3,
                                         op=SUB).then_inc(dve_sem)
                    in_o = bass.AP(xt, (KH + lo) * 3 * WX + i * WX,
                                   [[XP, 128], [3 * WX, kh], [2, 2], [1, ws]])
                    out_o = bass.AP(diff_t,
                                    lo * 3 * KW * WS + i * KW * WS + WS,
                                    [[DP, 128], [3 * KW * WS, kh],
                                     [2 * WS, 2], [1, ws]])
                    vector.tensor_tensor(out=out_o, in0=in_o, in1=cen2,
                                         op=SUB).then_inc(dve_sem)

            def d2adds(s, lo=0, hi=KH, small=False):
                kh, kw, ws = (1, 1, 2) if small else (hi - lo, KW, WS)
                d2 = bass.AP(d2_t[s % 2], lo * KW * WS,
                             [[GP, 128], [KW * WS, kh], [WS, kw], [1, ws]])
                sq_i = [bass.AP(sq_t, lo * 3 * KW * WS + i * KW * WS,
                                [[DP, 128], [3 * KW * WS, kh], [WS, kw],
                                 [1, ws]]) for i in range(3)]
                vector.tensor_tensor(out=d2, in0=sq_i[0], in1=sq_i[1],
                                     op=ADD).then_inc(dve_sem)
                vector.tensor_tensor(out=d2, in0=d2, in1=sq_i[2],
                                     op=ADD).then_inc(dve_sem)

            def mul_even(s, dy):
                st, g5 = sm_t[s % 2], g5_t[s % 2]
                tt = t_t[(5 * s + dy) % 3]
                de = dxs_even(dy)
                stride = de[1] - de[0]
                out_e = bass.AP(tt, de[0] * C * WS,
                                [[TP, 128], [stride * C * WS, len(de)],
                                 [WS, C], [1, WS]])
                sm_e = bass.AP(st, dy * C * WX,
                               [[SP, 128], [stride, len(de)], [WX, C],
                                [1, WS]])
                g_e = bass.AP(g5, dy * KW * WS + de[0] * WS,
                              [[GP, 128], [stride * WS, len(de)], [0, C],
                               [1, WS]])
                vector.tensor_tensor(out=out_e, in0=sm_e, in1=g_e,
                                     op=MULT).then_inc(dve_sem)

            def mul_odd(s, dy):
                st, g5 = sm_t[s % 2], g5_t[s % 2]
                tt = t_t[(5 * s + dy) % 3]
                out_o = bass.AP(tt, C * WS,
                                [[TP, 128], [2 * C * WS, 2], [WS, C],
                                 [1, WS]])
                sm_o = bass.AP(st, SE + dy * C * WX,
                               [[SP, 128], [2, 2], [WX, C], [1, WS]])
                g_o = bass.AP(g5, dy * KW * WS + WS,
                              [[GP, 128], [2 * WS, 2], [0, C], [1, WS]])
                vector.tensor_tensor(out=out_o, in0=sm_o, in1=g_o,
                                     op=MULT).then_inc(dve_sem)

            # step-0 prologue, pipelined per dy: dy0's chain first so
            # the first muls start ~10us earlier, dy1-4 overlap them
            vector.wait_ge(load_sem, 16)
            subs(0, 0, 1)
            vector.wait_ge(act_sem, 1)
            d2adds(0, 0, 1)
            subs(0, 1, KH)
            for s in range(NSTEP):
                for dy in range(KH):
                    # t buffer round-robin: block b=5s+dy reuses t[b%3],
                    # free once PE finished block b-3
                    pe_need = 15 * (5 * s + dy - 2)
                    if pe_need > 0:
                        vector.wait_ge(pe_sem, pe_need)
                    if dy == 0:
                        vector.wait_ge(act_sem, act_exp(s, 0))
                        if s == 0:
                            vector.wait_ge(sma_sem, 16)
                        else:
                            vector.wait_ge(sm0_sem, smc(s, dy))
                    elif s == 0 and dy in (1, 2):
                        if dy == 1:
                            vector.wait_ge(act_sem, act_exp(0, 1))
                            vector.wait_ge(sma_sem, 16)
                        else:
                            vector.wait_ge(sm0_sem, smc(s, dy))
                    mul_even(s, dy)
                    vector.wait_ge(act_sem, act_pcs(s, dy))
                    mul_odd(s, dy)
                    if dy == 0 and s == 0:
                        vector.wait_ge(act_sem, 4)
                        d2adds(0, 1, KH)
                    elif dy == 1:
                        if s + 1 < NSTEP:
                            vector.wait_ge(load_sem, 16 * (s + 2))
                            subs(s + 1)
                        else:
                            subs(s, small=True)
                    elif dy == 3:
                        if s + 1 < NSTEP:
                            vector.wait_ge(act_sem, act_sq(s + 1))
                            d2adds(s + 1)
                        else:
                            d2adds(s, small=True)

        @block.scalar
        def _(scalar):
            EXP = mybir.ActivationFunctionType.Exp
            SQR = mybir.ActivationFunctionType.Square
            CPY = mybir.ActivationFunctionType.Copy

            def pcs(s, dy):
                # on-device odd-parity copy: e1[dy,c,0:66] = e0[dy,c,1:67]
                # (e1 reuse guard vs step s-2 odd muls is implied by the
                # sq(s) dve wait that precedes every pcs in program order)
                if s == 0 and dy < 2:
                    scalar.wait_ge(sma_sem, 16)
                else:
                    scalar.wait_ge(sm0_sem, smc(s, dy))
                b = s % 2
                scalar.activation(
                    out=bass.AP(sm_t[b], SE + dy * C * WX,
                                [[SP, 128], [WX, C], [1, WX - 2]]),
                    in_=bass.AP(sm_t[b], dy * C * WX + 1,
                                [[SP, 128], [WX, C], [1, WX - 2]]),
                    func=CPY).then_inc(act_sem)

            def sq(s):
                scalar.wait_ge(dve_sem, dve_at(s - 1, 10) if s else 6)
                scalar.activation(
                    out=bass.AP(sq_t, 0, [[DP, 128], [1, DP]]),
                    in_=bass.AP(diff_t, 0, [[DP, 128], [1, DP]]),
                    func=SQR).then_inc(act_sem)

            def exp(s):
                scalar.wait_ge(dve_sem,
                               dve_at(s - 1, 16) if s else DVE_PRO)
                scalar.activation(
                    out=bass.AP(g5_t[s % 2], 0, [[GP, 128], [1, GP]]),
                    in_=bass.AP(d2_t[s % 2], 0, [[GP, 128], [1, GP]]),
                    func=EXP, scale=-0.5).then_inc(act_sem)

            # step-0: per-dy split (sq/exp for dy0 at once, dy1-4 after)
            scalar.wait_ge(dve_sem, 6)
            scalar.activation(
                out=bass.AP(sq_t, 0, [[DP, 128], [1, 3 * KW * WS]]),
                in_=bass.AP(diff_t, 0, [[DP, 128], [1, 3 * KW * WS]]),
                func=SQR).then_inc(act_sem)
            scalar.wait_ge(dve_sem, 8)
            scalar.activation(
                out=bass.AP(g5_t[0], 0, [[GP, 128], [1, KW * WS]]),
                in_=bass.AP(d2_t[0], 0, [[GP, 128], [1, KW * WS]]),
                func=EXP, scale=-0.5).then_inc(act_sem)
            pcs(0, 0)
            scalar.wait_ge(dve_sem, 14)
            scalar.activation(
                out=bass.AP(sq_t, 3 * KW * WS,
                            [[DP, 128], [1, DP - 3 * KW * WS]]),
                in_=bass.AP(diff_t, 3 * KW * WS,
                            [[DP, 128], [1, DP - 3 * KW * WS]]),
                func=SQR).then_inc(act_sem)
            pcs(0, 1)
            scalar.wait_ge(dve_sem, 18)
            scalar.activation(
                out=bass.AP(g5_t[0], KW * WS, [[GP, 128], [1, GP - KW * WS]]),
                in_=bass.AP(d2_t[0], KW * WS, [[GP, 128], [1, GP - KW * WS]]),
                func=EXP, scale=-0.5).then_inc(act_sem)
            pcs(0, 2)
            pcs(0, 3)
            pcs(0, 4)
            for s in range(NSTEP):
                if s + 1 < NSTEP:
                    sq(s + 1)
                    exp(s + 1)
                    pcs(s + 1, 0)
                    pcs(s + 1, 1)
                scalar.wait_ge(pe_sem, 75 * (s + 1))
                if s >= 2:
                    scalar.wait_ge(store_sem, 16 * (s - 1))
                for lo, ln in CT:
                    scalar.activation(
                        out=bass.AP(out_t[s % 2], lo, [[OP, 128], [1, ln]]),
                        in_=ps_t[s % 2].ap()[:, lo:lo + ln],
                        func=CPY).then_inc(drain_sem)
                if s + 1 < NSTEP:
                    pcs(s + 1, 2)
                    pcs(s + 1, 3)
                    pcs(s + 1, 4)

        @block.tensor
        def _(tensor):
            tensor.wait_ge(id_sem, 16)
            lhsT = id_t.ap()

            def mm(s, dy, dxs, start, stop):
                for lo, ln in CT:
                    rhs = bass.AP(t_t[(5 * s + dy) % 3],
                                  dxs * C * WS + lo, [[TP, 128], [1, ln]])
                    tensor.matmul(
                        ps_t[s % 2].ap()[:, lo:lo + ln], lhsT, rhs,
                        start=start, stop=stop,
                        skip_group_check=True).then_inc(pe_sem)

            for s in range(NSTEP):
                for dy in range(KH):
                    tensor.wait_ge(dve_sem, dve_at(s, POS_EVEN[dy]))
                    if dy == 0 and s >= 1:
                        tensor.wait_ge(drain_sem, 3 * (s - 1))
                    for dxs in dxs_even(dy):
                        mm(s, dy, dxs, start=(dy == 0 and dxs == 0),
                           stop=False)
                    if dy == 2:
                        # center term: g == 1, accumulate sm directly
                        tensor.wait_ge(sm0_sem, smc(s, 2))
                        for lo, ln in CT:
                            c0, ncl = lo // WS, ln // WS
                            rhs = bass.AP(sm_t[s % 2],
                                          2 * C * WX + c0 * WX + PAD,
                                          [[SP, 128], [WX, ncl], [1, WS]])
                            tensor.matmul(
                                ps_t[s % 2].ap()[:, lo:lo + ln], lhsT, rhs,
                                start=False, stop=False,
                                skip_group_check=True).then_inc(pe_sem)
                    tensor.wait_ge(dve_sem, dve_at(s, POS_ODD[dy]))
                    for dxs in (1, 3):
                        mm(s, dy, dxs, start=False,
                           stop=(dy == 4 and dxs == 3))

    return nc


def _prep_core(xyz, softmax, mask, core):
    """Host-side slab bake (bf16): fold mask into softmax, pad, and lay
    out dy-shifted windows per partition row (xyz also parity-duplicated;
    sm's +1 parity copy is built on-device)."""
    n, half = core // 2, core % 2
    w0 = WCORE * half
    wp_sz = WCORE + 2 * PAD + 1
    lo, hi = w0 - PAD, w0 + WCORE + PAD + 1
    glo, ghi = max(lo, 0), min(hi, W)

    sm_m = (softmax[n][:, :, glo:ghi] *
            mask[n][None, :, glo:ghi]).astype(BF16)
    smp = np.zeros((HH, C, wp_sz), BF16)
    smp[PAD:PAD + H, :, glo - lo:ghi - lo] = sm_m.transpose(1, 0, 2)
    xyzp = np.zeros((HH, 3, wp_sz), BF16)
    xyzp[PAD:PAD + H, :, glo - lo:ghi - lo] = (
        xyz[n][:, :, glo:ghi].transpose(1, 0, 2).astype(BF16))

    sm5 = np.empty((NSTEP, 128, KH, C, WX), BF16)
    xyz5 = np.empty((NSTEP, 128, 2, KH, 3, WX), BF16)
    for s in range(NSTEP):
        for chunk in range(2):
            wb = WS * (s + NSTEP * chunk)
            pr = slice(64 * chunk, 64 * chunk + 64)
            for dy in range(KH):
                sm5[s, pr, dy] = smp[dy:dy + 64, :, wb:wb + WX]
                for e in range(2):
                    xyz5[s, pr, e, dy] = xyzp[dy:dy + 64, :,
                                              wb + e:wb + e + WX]
    ident = np.eye(128, dtype=BF16)
    return {"sm_in": sm5, "xyz_in": xyz5, "ident_in": ident}


def make_in_maps(xyz, softmax, mask):
    return [_prep_core(xyz, softmax, mask, k) for k in range(8)]


def assemble_out(results):
    out = np.empty((N, C, H, W), np.float32)
    for core in range(8):
        n, half = core // 2, core % 2
        w0 = WCORE * half
        o = np.asarray(results[core]["out_d"]).astype(np.float32)
        # [s, chunk*64+h, c, j] -> [c, h, chunk*512 + s*64 + j]
        o = o.reshape(NSTEP, 2, H, C, WS)
        out[n, :, :, w0:w0 + WCORE] = o.transpose(3, 2, 1, 0, 4).reshape(
            C, H, WCORE)
    return out


def get_nc():
    if "nc" not in _CACHE:
        _CACHE["nc"] = _build_nc()
    return _CACHE["nc"]


def kernel(xyz, softmax, mask, trace=False, trace_kwargs=None):
    nc = get_nc()
    in_maps = make_in_maps(np.asarray(xyz), np.asarray(softmax),
                           np.asarray(mask))
    res = run_bass_kernel_spmd(nc, in_maps, list(range(8)), trace=trace,
                               **(trace_kwargs or {}))
    out = assemble_out(res.results)
    if trace:
        return out, res
    return out
